# revision 2
# baseline (speedup 1.0000x reference)
"""Trainium2 Bass kernel for nn_MoELayer (moe_routing).

Expert-parallel across 8 NeuronCores: the host computes the replicated gate
(~0.4% of FLOPs) and dispatches each token row to the cores owning its two
selected experts; each core runs its expert's MLP over its routed slots; the
host combine gathers each token's two slots and adds the b2 bias terms.

Load balancing: a core's slot space is a static primary region for its own
expert plus one static secondary region with its own weight-blob input;
experts with more than `acap` routed tokens spill overflow into other cores'
secondary regions (host-chosen assignment; the program is identical on every
core), so per-core work is ~mean load, not worst-expert load.

GEMM1 runs as compensated fp8 (e4m3) in DoubleRow perf mode: the PE
processes both 128-deep k-tiles of the C=256 contraction per pass at 0.5
cycles/row (4x bf16 throughput per pass), and three passes

    h = W1q.T @ x_hi  +  W1q.T @ x_lo  +  R1.T @ x_hi

(x_hi = e4m3(x), x_lo = e4m3(x - x_hi), W1q = e4m3(W1), R1 = e4m3(W1 - W1q))
recover ~bf16 accuracy at 6 PE-cycles/slot vs bf16's 8.  GEMM2 stays bf16
([cout, slot] orientation): quantizing GELU outputs to a single fp8 costs
~2.1e-2 end-to-end error, over the 2e-2 gate, and the fp8 residual of h
would need an extra elementwise pass on an already-busy engine.

Per core, per 512-slot chunk: GEMM1 (3 DR passes x 4 h-tiles) -> exact GELU
+ b1 (ACT, per-partition fused bias) -> GEMM2 (PE, bf16) -> multiply by the
token's normalized top-2 gate weight (DVE, host-replicated weight row) ->
bf16 slot outputs.

Cost-model-guided details (TimelineSim is the reference):
  - ~24 warmup matmuls on a zeroed tile keep PE busy from t~1us so the
    clock-ramp model reaches peak (2.4 GHz) before the first real GEMM;
  - a small 384-col leading chunk starts real compute as soon as the first
    x DMA lands;
  - DMA issue order is arrival order == need order;
  - software pipeline: GEMM1(c+1) issues before GEMM2(c) so GELU overlaps;
  - the secondary chunk runs last (its weights arrive mid-kernel); all
    slot-trailing small units share one output tile flushed by a single
    final DMA whose layout keeps >=512B runs.

Layouts (P=128 partitions):
  xth/xtl [P, KC, CAP] f8e4  xth[p,k,s] = e4m3(x)_slot[s, 128k+p]; xtl the
                             residual e4m3(x - x_hi)
  wq1/wr1 [P, 1024]    f8e4  col hc*256+two*128+m = W1q[e][two*128+p, hc*128+m]
  bias    [P, (1+NSEC)*KH] bf16  b1 wrapped [KH, P].T, primary then secondaries
  w2      [P, 1024]    bf16  cols hc*256+o = W2[e, 128hc+p, o]
  wbs8    [P, NSEC, 2048]  f8e4  secondary expert wq1-layout ++ wr1-layout
  wbs16   [P, NSEC, 1024]  bf16  secondary expert w2-layout
  wrep    [P, CAP]     bf16  wrep[p, s] = normalized gate weight of slot s
  out     [P, 2, CAP]  bf16  out[p,ct,s] = w_s * GEMM2[ct*128+p, s]
"""

import os
import sys

sys.path.insert(0, "/opt/trn_rl_repo")
os.environ.setdefault("JAX_PLATFORMS", "")
os.environ.setdefault("NEURON_RT_RESET_CORES", "1")

import numpy as np
import ml_dtypes

B, M, H, W, C = 2, 4, 32, 32, 256
E, TOPK, HID, C_OUT = 8, 2, 512, 256
T = B * M * H * W          # 8192 tokens
NCORES = 8
P = 128
KC = C // P                # 2 k-subtiles over C (the DoubleRow pair dim)
KH = HID // P              # 4 k-subtiles over HID
NCT = C_OUT // P           # 2 output-column tiles
NCHUNK = 512               # moving-dim chunk (one PSUM bank at fp32)
ATILES = 16                # primary region tiles (2048 slots)
NSEC = 1                   # secondary 128-slot regions per core
W1COLS = KH * 2 * P        # 1024 cols per W1 pair-layout blob

_BUILD_CACHE = {}

DEFAULT_CFG = dict(
    gelu_pair=False,
    depth=2,          # G1 units issued ahead of each unit's G2
    psh_bufs=5,
    psy_bufs=3,
    ht_bufs=3,
    yo_bufs=4,
    out_pool=False,   # SP-issued output DMAs (Pool DGE costs more)
    nwarm=24,
)


def _chunks(acap, split0=False):
    """Primary-region chunks: [384, 512, ..., remainder].  384 first: large
    enough to keep PE busy while the next x chunk transfers, small enough to
    start early; the small remainder lands at the tail (cheap final DMA)."""
    out = []
    off = 0
    if acap >= 384 + NCHUNK and acap % NCHUNK == 0:
        if split0:
            out += [(0, 192), (192, 192)]
        else:
            out.append((0, 384))
        off = 384
    for _ in range((acap - off) // NCHUNK):
        out.append((off, NCHUNK))
        off += NCHUNK
    if acap - off:
        out.append((off, acap - off))
    return out


def _build(atiles, nsec, cfg=None, sec_w=P):
    import concourse.bacc as bacc
    import concourse.mybir as mybir
    from concourse.tile import TileContext

    cfg = dict(DEFAULT_CFG, **(cfg or {}))
    dt = mybir.dt
    AF = mybir.ActivationFunctionType
    OP = mybir.AluOpType
    PM = mybir.MatmulPerfMode

    acap = atiles * P
    secr = -(-sec_w // P) * P          # P-aligned secondary region stride
    cap = acap + nsec * secr
    chunks = _chunks(acap, cfg.get("split0", False))
    sec_chunks = [(acap + s * secr, sec_w) for s in range(nsec)]

    nc = bacc.Bacc("TRN2", target_bir_lowering=False)

    xth_d = nc.dram_tensor("xth", [P, KC, cap], dt.float8e4, kind="ExternalInput")
    xtl_d = nc.dram_tensor("xtl", [P, KC, cap], dt.float8e4, kind="ExternalInput")
    wq1_d = nc.dram_tensor("wq1", [P, W1COLS], dt.float8e4, kind="ExternalInput")
    wr1_d = nc.dram_tensor("wr1", [P, W1COLS], dt.float8e4, kind="ExternalInput")
    bias_d = nc.dram_tensor("bias", [P, (1 + nsec) * KH], dt.bfloat16,
                            kind="ExternalInput")
    w2_d = nc.dram_tensor("w2", [P, KH * C_OUT], dt.bfloat16, kind="ExternalInput")
    wbs8_d = nc.dram_tensor("wbs8", [P, nsec, 2 * W1COLS], dt.float8e4,
                            kind="ExternalInput")
    wbs16_d = nc.dram_tensor("wbs16", [P, nsec, KH * C_OUT], dt.bfloat16,
                             kind="ExternalInput")
    wrep_d = nc.dram_tensor("wrep", [P, cap], dt.bfloat16, kind="ExternalInput")
    out_d = nc.dram_tensor("out", [P, NCT, cap], dt.bfloat16, kind="ExternalOutput")

    with TileContext(nc) as tc:
        with (
            tc.tile_pool(name="const", bufs=1) as cpool,
            tc.tile_pool(name="ht", bufs=cfg["ht_bufs"]) as htpool,
            tc.tile_pool(name="yo", bufs=cfg["yo_bufs"]) as ypool,
            tc.tile_pool(name="psh", bufs=cfg["psh_bufs"], space="PSUM") as psh,
            tc.tile_pool(name="psy", bufs=cfg["psy_bufs"], space="PSUM") as psy,
        ):
            # -------- PE warmup: keep the clock-ramp model hot ----------
            ps_wu = psy.tile([P, NCHUNK], dt.float32, tag="y", name="ps_wu")
            wu = cpool.tile([P, P], dt.bfloat16)
            nc.gpsimd.memset(wu[:], 0.0)
            ps_w = ps_wu[:, :P]
            for _ in range(cfg["nwarm"]):
                nc.tensor.matmul(ps_w[:], lhsT=wu[:], rhs=wu[:], start=True, stop=True)

            # -------- inputs (issue order == need order) ----------------
            xth_sb = cpool.tile([P, KC, cap], dt.float8e4)
            xtl_sb = cpool.tile([P, KC, cap], dt.float8e4)
            wq1_sb = cpool.tile([P, W1COLS], dt.float8e4)
            wr1_sb = cpool.tile([P, W1COLS], dt.float8e4)
            bias_sb = cpool.tile([P, (1 + nsec) * KH], dt.bfloat16)

            def dma_x(i):
                off, ncw = (chunks + sec_chunks)[i]
                nc.sync.dma_start(
                    xth_sb[:, :, off:off + ncw], xth_d[:, :, off:off + ncw]
                )
                nc.sync.dma_start(
                    xtl_sb[:, :, off:off + ncw], xtl_d[:, :, off:off + ncw]
                )

            dma_x(0)
            nc.sync.dma_start(wq1_sb[:], wq1_d[:])
            nc.sync.dma_start(wr1_sb[:], wr1_d[:])
            nc.sync.dma_start(bias_sb[:], bias_d[:])
            dma_x(1)
            w2_sb = cpool.tile([P, KH * C_OUT], dt.bfloat16)
            nc.sync.dma_start(w2_sb[:], w2_d[:])
            dma_x(2)
            wrep_sb = cpool.tile([P, cap], dt.bfloat16)
            nc.sync.dma_start(wrep_sb[:], wrep_d[:])
            for i in range(3, len(chunks) + len(sec_chunks)):
                dma_x(i)
            wbs8_sb = cpool.tile([P, nsec, 2 * W1COLS], dt.float8e4)
            nc.sync.dma_start(wbs8_sb[:], wbs8_d[:])
            wbs16_sb = cpool.tile([P, nsec, KH * C_OUT], dt.bfloat16)
            nc.sync.dma_start(wbs16_sb[:], wbs16_d[:])

            # -------- expert MLP ----------------------------------------
            def gemm1_unit(unit, wqap, wrap, bias_base):
                """3-pass compensated fp8 DoubleRow GEMM1 + GELU."""
                nhalf = len(unit)
                hT = htpool.tile([P, KH, nhalf, NCHUNK], dt.bfloat16, tag="hT")
                for hc in range(KH):
                    wq_pair = wqap[:, hc * 2 * P:(hc + 1) * 2 * P].rearrange(
                        "p (two f) -> p two f", two=2)
                    wr_pair = wrap[:, hc * 2 * P:(hc + 1) * 2 * P].rearrange(
                        "p (two f) -> p two f", two=2)
                    ps_h = psh.tile([P, nhalf, NCHUNK], dt.float32, tag="h")
                    for half, (off, ncw) in enumerate(unit):
                        xh = xth_sb[:, :, off:off + ncw]
                        xl = xtl_sb[:, :, off:off + ncw]
                        nc.tensor.matmul(
                            ps_h[:, half, :ncw], lhsT=wq_pair, rhs=xh,
                            start=True, stop=False, perf_mode=PM.DoubleRow,
                        )
                        nc.tensor.matmul(
                            ps_h[:, half, :ncw], lhsT=wq_pair, rhs=xl,
                            start=False, stop=False, perf_mode=PM.DoubleRow,
                        )
                        nc.tensor.matmul(
                            ps_h[:, half, :ncw], lhsT=wr_pair, rhs=xh,
                            start=False, stop=True, perf_mode=PM.DoubleRow,
                        )
                    ncw0 = unit[0][1]
                    bcol = bias_base + hc
                    nc.scalar.activation(
                        hT[:, hc, :nhalf, :ncw0], ps_h[:, :nhalf, :ncw0],
                        AF.Gelu,
                        bias=bias_sb[:, bcol:bcol + 1],
                    )
                return hT

            dma_out = nc.gpsimd.dma_start if cfg["out_pool"] else nc.sync.dma_start

            def gemm2_half(hT, w2ap, half, off, ncw, ytail=None, tpos=0,
                           last_unit=False):
                if ytail is not None and ncw <= NCHUNK // NCT:
                    # both column tiles in one PSUM bank: one DVE op; the
                    # output lands in the shared tail tile (DMA'd once at
                    # the end) to avoid serialized tiny-DMA init chains.
                    ps_y = psy.tile([P, NCHUNK], dt.float32, tag="y")
                    for ct in range(NCT):
                        for hc in range(KH):
                            nc.tensor.matmul(
                                ps_y[:, ct * ncw:(ct + 1) * ncw],
                                lhsT=w2ap[:, hc * C_OUT + ct * P:hc * C_OUT + (ct + 1) * P],
                                rhs=hT[:, hc, half, :ncw],
                                start=(hc == 0),
                                stop=(hc == KH - 1),
                            )
                    nc.vector.tensor_tensor(
                        ytail[:, :, tpos:tpos + ncw],
                        ps_y[:, :NCT * ncw].rearrange("p (c n) -> p c n", c=NCT),
                        wrep_sb[:, None, off:off + ncw].to_broadcast([P, NCT, ncw]),
                        OP.mult,
                    )
                    return
                y_sb = ypool.tile([P, NCT, NCHUNK], dt.bfloat16, tag="y")
                for ct in range(NCT):
                    ps_y = psy.tile([P, NCHUNK], dt.float32, tag="y")
                    for hc in range(KH):
                        nc.tensor.matmul(
                            ps_y[:, :ncw],
                            lhsT=w2ap[:, hc * C_OUT + ct * P:hc * C_OUT + (ct + 1) * P],
                            rhs=hT[:, hc, half, :ncw],
                            start=(hc == 0),
                            stop=(hc == KH - 1),
                        )
                    nc.vector.tensor_tensor(
                        y_sb[:, ct, :ncw], ps_y[:, :ncw],
                        wrep_sb[:, off:off + ncw], OP.mult,
                    )
                dma_out(out_d[:, :, off:off + ncw], y_sb[:, :, :ncw])

            # units: primary chunks (optionally paired), then secondaries
            if cfg["gelu_pair"]:
                units = [[chunks[0]]]
                k = 1
                while k < len(chunks):
                    units.append(chunks[k:k + 2])
                    k += 2
            else:
                units = [[c] for c in chunks]
            units += [[sc] for sc in sec_chunks]
            nprim = len(units) - nsec
            if (cfg.get("tail_early", False) and nprim >= 2 and nsec
                    and units[nprim - 1][0][1] <= NCHUNK // NCT
                    and units[nprim - 2][0][1] > NCHUNK // NCT):
                # run the small primary-tail unit before the last big one so
                # its DVE scale overlaps the big unit's GEMM2
                units[nprim - 2], units[nprim - 1] = (
                    units[nprim - 1], units[nprim - 2])

            def weights_for(u):
                if u < nprim:
                    return wq1_sb[:], wr1_sb[:], w2_sb[:], 0
                s = u - nprim
                return (
                    wbs8_sb[:, s, :W1COLS],
                    wbs8_sb[:, s, W1COLS:],
                    wbs16_sb[:, s, :],
                    (1 + s) * KH,
                )

            # slot-trailing small units share one output tile + one final
            # DMA (membership by slot offset, independent of schedule order).
            tail_base = acap
            if chunks and chunks[-1][1] <= NCHUNK // NCT:
                tail_base = chunks[-1][0]
            include_sec = sec_w <= NCHUNK // NCT
            tail_end = cap if include_sec else acap
            tail_units = [u for u in units
                          if len(u) == 1 and tail_end > u[0][0] >= tail_base]
            ytail = None
            if tail_units:
                tail_w = tail_end - tail_base
                ytail = ypool.tile([P, NCT, tail_w], dt.bfloat16, tag="yt")
                # memset strips not covered by any unit's compute width
                cov = np.zeros(tail_w, dtype=bool)
                for (off, ncw), in [(u[0],) for u in tail_units]:
                    cov[off - tail_base:off - tail_base + ncw] = True
                j = 0
                while j < tail_w:
                    if not cov[j]:
                        j2 = j
                        while j2 < tail_w and not cov[j2]:
                            j2 += 1
                        nc.vector.memset(ytail[:, :, j:j2], 0.0)
                        j = j2
                    else:
                        j += 1

            depth = cfg["depth"]
            hts = {}
            for u in range(min(depth, len(units))):
                w_ = weights_for(u)
                hts[u] = gemm1_unit(units[u], w_[0], w_[1], w_[3])
            for u in range(len(units)):
                if u + depth < len(units):
                    v = u + depth
                    w_ = weights_for(v)
                    hts[v] = gemm1_unit(units[v], w_[0], w_[1], w_[3])
                w2ap = weights_for(u)[2]
                for half, (off, ncw) in enumerate(units[u]):
                    in_tail = (ytail is not None and len(units[u]) == 1
                               and tail_end > off >= tail_base)
                    gemm2_half(
                        hts[u], w2ap, half, off, ncw,
                        ytail=(ytail if in_tail else None),
                        tpos=off - tail_base if in_tail else 0,
                        last_unit=(u == len(units) - 1),
                    )
            if ytail is not None:
                dma_out(out_d[:, :, tail_base:tail_end], ytail[:])

    nc.compile()
    return nc


def _get_nc(atiles=ATILES, nsec=NSEC, cfg=None, sec_w=P):
    key = (atiles, nsec, sec_w, tuple(sorted((cfg or {}).items())))
    if key not in _BUILD_CACHE:
        _BUILD_CACHE[key] = _build(atiles, nsec, cfg, sec_w)
    return _BUILD_CACHE[key]


def _route(inputs):
    """Replicated gate on the host; top-2 routing + normalized weights."""
    x = np.asarray(inputs["x"], dtype=np.float32).reshape(T, C)
    logits = (
        x @ np.asarray(inputs["Wg"], dtype=np.float32)
        + np.asarray(inputs["bg"], dtype=np.float32)
        + np.asarray(inputs["expert_bias"], dtype=np.float32)
    )
    # top-2 (ties broken by lower index, matching jax.lax.top_k)
    idx = np.argsort(-logits, axis=1, kind="stable")[:, :TOPK]       # [T, 2]
    vals = np.take_along_axis(logits, idx, axis=1)                   # [T, 2]
    return x, logits, idx, vals


def _plan(idx):
    """Choose the (primary capacity, secondary width) pair minimizing total
    per-core compute (acap + sec_w) such that every expert's overflow packs
    into the NCORES*NSEC per-core secondary segments."""
    cnt = np.bincount(idx.ravel(), minlength=E)

    def min_secw(acap):
        for sec_w in range(32, 4 * P + 1, 32):
            pieces = sum(int(-(-max(0, c - acap) // sec_w)) for c in cnt)
            if pieces <= NCORES * NSEC:
                return sec_w
        return None

    best = None
    atiles = max(1, ATILES - 2)
    while True:
        acap = atiles * P
        if best is not None and acap + 32 >= best[0] * P + best[1]:
            return best
        sec_w = min_secw(acap)
        if sec_w is not None and (
            best is None
            or acap + sec_w < best[0] * P + best[1]
        ):
            best = (atiles, sec_w)
        atiles += 1


def _e4(a):
    return a.astype(ml_dtypes.float8_e4m3)


def _pack_w1(W1e):
    """Pair layout: col hc*256+two*128+m = W1[two*128+p, hc*128+m]; returns
    (quantized, residual) e4m3 blobs [P, 1024] each."""
    q = _e4(W1e)
    r = _e4(W1e - q.astype(np.float32))
    def lay(a):
        return np.ascontiguousarray(
            a.astype(np.float32).reshape(KC, P, KH, P).transpose(1, 2, 0, 3)
            .reshape(P, W1COLS)
        )
    return _e4(lay(q)), _e4(lay(r))


def _stage(inputs, x, logits, idx, vals, atiles, sec_w=P):
    """Build the 8 per-core input maps (dispatch by top-k index)."""
    W1 = np.asarray(inputs["W1"], dtype=np.float32)
    b1 = np.asarray(inputs["b1"], dtype=np.float32)
    W2 = np.asarray(inputs["W2"], dtype=np.float32)
    acap = atiles * P
    secr = -(-sec_w // P) * P
    cap = acap + NSEC * secr

    wgt = 1.0 / (1.0 + np.exp(-vals))
    wgt = wgt / wgt.sum(axis=1, keepdims=True)                       # [T, 2]

    # primary slots + overflow tile queue
    gpos = np.empty((T, TOPK), dtype=np.int64)   # (t, j) -> core * cap + slot
    prim = []                                    # per expert: primary tokens
    prim_j = []
    spill = []                                   # (expert, tokens, js)
    for e in range(E):
        te, je = np.nonzero(idx == e)
        prim.append(te[:acap]); prim_j.append(je[:acap])
        for s in range(acap, len(te), sec_w):
            spill.append((e, te[s:s + sec_w], je[s:s + sec_w]))
    assert all(len(t) <= sec_w for _, t, _ in spill)
    assert len(spill) <= NCORES * NSEC, "secondary capacity exceeded"

    w2p = {}
    for e in range(E):
        w2p[e] = np.ascontiguousarray(
            W2[e].reshape(KH, P, C_OUT).transpose(1, 0, 2).reshape(P, KH * C_OUT)
        ).astype(ml_dtypes.bfloat16)
    w1p = {e: _pack_w1(W1[e]) for e in range(E)}
    b1p = {e: np.ascontiguousarray(b1[e].reshape(KH, P).T) for e in range(E)}

    in_maps = []
    for c in range(NCORES):
        te, je = prim[c], prim_j[c]
        n = len(te)
        gpos[te, je] = c * cap + np.arange(n)

        xs = np.zeros((cap, C), dtype=np.float32)
        xs[:n] = x[te]
        wr = np.zeros((cap,), dtype=np.float32)
        wr[:n] = wgt[te, je]

        wbs8 = np.zeros((P, NSEC, 2 * W1COLS), dtype=ml_dtypes.float8_e4m3)
        wbs16 = np.zeros((P, NSEC, KH * C_OUT), dtype=ml_dtypes.bfloat16)
        bias = np.zeros((P, (1 + NSEC) * KH), dtype=ml_dtypes.bfloat16)
        bias[:, :KH] = b1p[c]
        for s in range(NSEC):
            qi = c * NSEC + s
            if qi < len(spill):
                se, ste, sje = spill[qi]
                m = len(ste)
                off = acap + s * secr
                xs[off:off + m] = x[ste]
                wr[off:off + m] = wgt[ste, sje]
                gpos[ste, sje] = c * cap + off + np.arange(m)
                wbs8[:, s, :W1COLS] = w1p[se][0]
                wbs8[:, s, W1COLS:] = w1p[se][1]
                wbs16[:, s, :] = w2p[se]
                bias[:, (1 + s) * KH:(2 + s) * KH] = b1p[se]

        xs_hi = _e4(xs)
        xs_lo = _e4(xs - xs_hi.astype(np.float32))
        def xlay(a):
            return np.ascontiguousarray(
                a.astype(np.float32).T.reshape(KC, P, cap).transpose(1, 0, 2)
            )
        in_maps.append({
            "xth": _e4(xlay(xs_hi)),
            "xtl": _e4(xlay(xs_lo)),
            "wq1": w1p[c][0],
            "wr1": w1p[c][1],
            "bias": bias,
            "w2": w2p[c],
            "wbs8": wbs8,
            "wbs16": wbs16,
            "wrep": np.broadcast_to(
                wr.astype(ml_dtypes.bfloat16), (P, cap)
            ).copy(),
        })
    return in_maps, gpos, cap


def _prepare(inputs):
    x, logits, idx, vals = _route(inputs)
    atiles, sec_w = _plan(idx)
    nc = _get_nc(atiles, NSEC, sec_w=sec_w)
    in_maps, gpos, cap = _stage(inputs, x, logits, idx, vals, atiles, sec_w)
    return nc, in_maps, gpos, cap, idx, vals


def kernel(**inputs):
    from concourse.bass_utils import run_bass_kernel_spmd

    nc, in_maps, gpos, cap, idx, vals = _prepare(inputs)
    res = run_bass_kernel_spmd(nc, in_maps, core_ids=list(range(NCORES)))

    # all-to-all combine: out[t] = y[slot(t,0)] + y[slot(t,1)] + comb @ b2
    y = np.empty((NCORES * cap, C_OUT), dtype=np.float32)
    for c in range(NCORES):
        yc = np.asarray(res.results[c]["out"], dtype=np.float32)  # [P, NCT, cap]
        y[c * cap:(c + 1) * cap] = yc.transpose(2, 1, 0).reshape(cap, C_OUT)

    b2 = np.asarray(inputs["b2"], dtype=np.float32)
    wgt = 1.0 / (1.0 + np.exp(-vals))
    wgt = wgt / wgt.sum(axis=1, keepdims=True)
    out = (
        y[gpos[:, 0]] + y[gpos[:, 1]]
        + wgt[:, 0:1] * b2[idx[:, 0]] + wgt[:, 1:2] * b2[idx[:, 1]]
    )
    return out.reshape(B, M, H, W, C_OUT).astype(np.float32)
